# revision 6
# baseline (speedup 1.0000x reference)
"""Weighted cross-entropy loss (nn_CustomCrossEntropyLoss) on 8 Trainium2 NeuronCores.

Strategy (data-parallel, per sharding hint): shard the N=4M rows across the 8
cores; each core computes a partial weighted-loss sum and nonzero count fully
on-device (log-softmax + target gather + weighted reduction); host combines the
16 partial scalars.

Per-core layout: rows are packed row-major into T tiles of [128 partitions, F
rows, 9 classes].  Per tile:
  ACT:  E = exp(X)                     (no max-subtraction needed: |x| < 6)
  DVE:  S = segmented_reduce(E, 9)     -> [128, F]
  ACT:  L = ln(S)                      (= logsumexp per row)
  DVE:  weighted one-hot masks M_c = (t == c) * w_c   (dual-op tensor_scalar)
        XT = gather of target logit    (copy_predicated chain over classes)
        WT = sum_c M_c                 (= w[t]; 0 for pad rows with t=9)
        D = L - XT; LOSS = WT*D  (+ per-partition accumulation via accum_out)
        CNT += (LOSS > 1e-16)
Pad rows use t=9 so every mask is 0 -> WT=0 -> LOSS=0 exactly (excluded from
both sum and count).
"""

import sys

if "/opt/trn_rl_repo" not in sys.path:
    sys.path.insert(0, "/opt/trn_rl_repo")

import numpy as np

import concourse.bass as bass
import concourse.mybir as mybir
from concourse.bass_utils import run_bass_kernel_spmd

F32 = mybir.dt.float32
AF = mybir.ActivationFunctionType
ALU = mybir.AluOpType

N = 4_000_000
C = 9
NCORES = 8
P = 128
T = 4          # tiles per core
F = 977        # rows per partition per tile; 8*128*T*F = 4_001_792 >= N
ROWS_PER_CORE = P * T * F
PAD = NCORES * ROWS_PER_CORE - N

W = [0.03203128, 0.12453853, 0.12360233, 0.12430233, 0.1118631,
     0.11928928, 0.12498565, 0.12078846, 0.11859904]

_CACHED = {}


def _build_nc():
    nc = bass.Bass()
    x = nc.declare_dram_parameter("x", [P, T, F * C], F32, isOutput=False)
    tg = nc.declare_dram_parameter("t", [P, T, F], F32, isOutput=False)
    y = nc.declare_dram_parameter("y", [P, 2], F32, isOutput=True)

    with (
        nc.sbuf_tensor([P, 2, F * C], F32) as Xb,
        nc.sbuf_tensor([P, 2, F * C], F32) as Eb,
        nc.sbuf_tensor([P, 2, F], F32) as Tb,
        nc.sbuf_tensor([P, 2, F], F32) as Sb,
        nc.sbuf_tensor([P, 2, F], F32) as Lb,
        nc.sbuf_tensor([P, F], F32) as Mb,
        nc.sbuf_tensor([P, F], F32) as XTb,
        nc.sbuf_tensor([P, F], F32) as WTb,
        nc.sbuf_tensor([P, F], F32) as LOSSb,
        nc.sbuf_tensor([P, F], F32) as ONESb,
        nc.sbuf_tensor([P, T], F32) as losscols,
        nc.sbuf_tensor([P, T], F32) as cntcols,
        nc.sbuf_tensor([P, 2], F32) as outb,
        nc.semaphore() as ES,
        nc.semaphore() as RS,
        nc.semaphore() as LS,
        nc.semaphore() as DN,
        nc.semaphore() as FIN,
        nc.semaphore() as DOUT,
    ):
        dx = [nc.semaphore(name=f"dx{_k}").__enter__() for _k in range(T)]

        def x3d(k):
            return Xb[:, k % 2, :].rearrange("p (f c) -> p f c", c=C)

        def e3d(k):
            return Eb[:, k % 2, :].rearrange("p (f c) -> p f c", c=C)

        with nc.Block() as block:

            @block.sync
            def _(sync):
                for k in range(T):
                    if k >= 2:
                        sync.wait_ge(DN, k - 1)
                    sync.dma_start(Xb[:, k % 2, :], x[:, k, :]).then_inc(dx[k], 16)
                    sync.dma_start(Tb[:, k % 2, :], tg[:, k, :]).then_inc(dx[k], 16)
                sync.wait_ge(FIN, 1)
                sync.dma_start(y[:, :], outb[:, :]).then_inc(DOUT, 16)
                sync.wait_ge(DOUT, 16)

            @block.scalar
            def _(scalar):
                for k in range(T):
                    scalar.wait_ge(dx[k], 32)
                    if k >= 2:
                        scalar.wait_ge(RS, k - 1)  # E slot free
                    scalar.activation(Eb[:, k % 2, :], Xb[:, k % 2, :], AF.Exp).then_inc(ES, 1)
                    scalar.wait_ge(RS, k + 1)
                    if k >= 2:
                        scalar.wait_ge(DN, k - 1)  # L slot free
                    scalar.activation(Lb[:, k % 2, :], Sb[:, k % 2, :], AF.Ln).then_inc(LS, 1)

            @block.vector
            def _(vector):
                vector.memset(ONESb[:, :], 1.0)
                for k in range(T):
                    s = k % 2
                    vector.wait_ge(ES, k + 1)
                    vector.tensor_reduce(
                        Sb[:, s, :], e3d(k), axis=mybir.AxisListType.X, op=ALU.add
                    ).then_inc(RS, 1)
                    # gather target logit and weight via weighted one-hot masks
                    vector.tensor_copy(XTb[:, :], x3d(k)[:, :, 0])
                    vector.tensor_scalar(WTb[:, :], Tb[:, s, :], 0.0, W[0], ALU.is_equal, ALU.mult)
                    for c in range(1, C):
                        vector.tensor_scalar(Mb[:, :], Tb[:, s, :], float(c), W[c], ALU.is_equal, ALU.mult)
                        vector.copy_predicated(
                            XTb[:, :], Mb[:, :].bitcast(mybir.dt.int32), x3d(k)[:, :, c]
                        )
                        vector.tensor_tensor(WTb[:, :], WTb[:, :], Mb[:, :], ALU.add)
                    vector.wait_ge(LS, k + 1)
                    # D = L - XT (reuse Mb)
                    vector.scalar_tensor_tensor(
                        Mb[:, :], XTb[:, :], -1.0, Lb[:, s, :], ALU.mult, ALU.add
                    )
                    # LOSS = WT * D ; losscols[:, k] = sum_f LOSS
                    vector.scalar_tensor_tensor(
                        LOSSb[:, :], WTb[:, :], 1.0, Mb[:, :], ALU.mult, ALU.mult,
                        accum_out=losscols[:, k : k + 1],
                    )
                    # cntcols[:, k] = sum_f (LOSS > 1e-16)
                    vector.scalar_tensor_tensor(
                        Mb[:, :], LOSSb[:, :], 1e-16, ONESb[:, :], ALU.is_gt, ALU.mult,
                        accum_out=cntcols[:, k : k + 1],
                    ).then_inc(DN, 1)
                vector.tensor_reduce(
                    outb[:, 0:1], losscols[:, :], axis=mybir.AxisListType.X, op=ALU.add
                )
                vector.tensor_reduce(
                    outb[:, 1:2], cntcols[:, :], axis=mybir.AxisListType.X, op=ALU.add
                ).then_inc(FIN, 1)

    return nc


def _get_nc():
    if "nc" not in _CACHED:
        _CACHED["nc"] = _build_nc()
    return _CACHED["nc"]


def _prep_inputs(logits, target):
    logits = np.asarray(logits, dtype=np.float32)
    target = np.asarray(target)
    xall = np.concatenate([logits, np.zeros((PAD, C), dtype=np.float32)], axis=0)
    tall = np.concatenate(
        [target.astype(np.float32), np.full((PAD,), 9.0, dtype=np.float32)]
    )
    xsh = xall.reshape(NCORES, P, T, F * C)
    tsh = tall.reshape(NCORES, P, T, F)
    return [{"x": xsh[i], "t": tsh[i]} for i in range(NCORES)]


def run_on_hw(logits, target, trace=False):
    nc = _get_nc()
    in_maps = _prep_inputs(logits, target)
    res = run_bass_kernel_spmd(nc, in_maps, core_ids=list(range(NCORES)), trace=trace)
    ys = np.stack([res.results[i]["y"] for i in range(NCORES)])  # [8, 128, 2]
    loss_sum = ys[:, :, 0].sum(dtype=np.float64)
    cnt = ys[:, :, 1].sum(dtype=np.float64)
    return loss_sum, cnt, res


def kernel(logits, target, class_weights=None):
    loss_sum, cnt, _ = run_on_hw(logits, target)
    out1 = np.float32(loss_sum / (cnt + 1e-16))
    out2 = np.float32(loss_sum / N)
    return (out1, out2)


if __name__ == "__main__":
    rng = np.random.default_rng(0)
    lg = rng.standard_normal((N, C), dtype=np.float32)
    tg = rng.integers(0, C, size=(N,)).astype(np.int64)
    print(kernel(lg, tg))
